# revision 1
# baseline (speedup 1.0000x reference)
"""Trainium2 Bass kernel for ChannelAttentionModel (segment avg/max -> tiny MLP ->
sigmoid gate -> per-point scale), SPMD across 8 NeuronCores.

Sharding: batch_ids is sorted with B=16 segments; core k owns batches 2k and
2k+1 (whole batches per device). Each batch range is padded to a fixed R points
by replicating the first row of the batch (max-safe); the extra rows' sum
contribution is subtracted via a host-computed correction term.
"""

import sys

for _p in ("/opt/trn_rl_repo", "/root/.axon_site/_ro/trn_rl_repo"):
    if _p not in sys.path:
        sys.path.append(_p)

import numpy as np

import concourse.bacc as bacc
import concourse.tile as tile
from concourse import bass, mybir
from concourse.bass_utils import run_bass_kernel_spmd
from concourse.masks import make_identity

NCORES = 8
B = 16
C = 64
H = 32
RPC = 2  # batch ranges per core
TP = 4096  # points per tile
FA = TP // 128  # free-dim point groups per partition (32)
F = FA * C  # free elems per partition per tile (2048)
DT = mybir.dt.float32


def build_nc(R: int, reps: int = 1, loop_reps: int = 1,
             do_add: bool = True, do_max: bool = True, do_mul: bool = True,
             skip_stats: bool = False, dma_engines: int = 1,
             do_phase1: bool = True, do_phase2: bool = True,
             chunk_tiles: int = 2, xbufs: int = 6, style: str = 'pe',
             max_gps_every: int = 0, inplace: bool = True,
             out_scalar: bool = False):
    NT = R // TP
    nc = bacc.Bacc("TRN2", target_bir_lowering=False, debug=False,
                   num_devices=NCORES, enable_asserts=False)

    xs = nc.dram_tensor("xs", [RPC, R, C], DT, kind="ExternalInput")
    corrt = nc.dram_tensor("corrt", [C, RPC], DT, kind="ExternalInput")
    invct = nc.dram_tensor("invct", [C, RPC], DT, kind="ExternalInput")
    w1t = nc.dram_tensor("w1t", [C, H], DT, kind="ExternalInput")
    b1c = nc.dram_tensor("b1c", [H, 1], DT, kind="ExternalInput")
    w2t = nc.dram_tensor("w2t", [H, C], DT, kind="ExternalInput")
    b2x2 = nc.dram_tensor("b2x2", [C, 1], DT, kind="ExternalInput")
    out = nc.dram_tensor("out", [RPC, R, C], DT, kind="ExternalOutput")

    def dram_chunk_ap(handle, r, off, npts):
        return handle.ap()[r, off:off + npts, :].rearrange(
            "(p a) c -> p (a c)", p=128)

    # chunk each range into large DMA transfers
    chunks = []
    off = 0
    while off < R:
        npts = min(chunk_tiles * TP, R - off)
        chunks.append((off, npts))
        off += npts

    with tile.TileContext(nc) as tc:
        with (
            tc.tile_pool(name="const", bufs=1) as const,
            tc.tile_pool(name="xpool", bufs=xbufs) as xpool,
            tc.tile_pool(name="accs", bufs=1) as accs,
            tc.tile_pool(name="stat", bufs=4) as stat,
            tc.tile_pool(name="small", bufs=1) as small,
            tc.tile_pool(name="psum_t", bufs=1, space="PSUM") as psum_t,
            tc.tile_pool(name="psum_w", bufs=1, space="PSUM") as psum_w,
        ):
            # constants
            ident = const.tile([128, 128], DT)
            make_identity(nc, ident[:])
            ones_row = const.tile([1, 128], DT)
            nc.vector.memset(ones_row[:], 1.0)
            ones_bf = const.tile([128, 1], mybir.dt.bfloat16)
            nc.vector.memset(ones_bf[:], 1.0)
            corrt_sb = const.tile([C, RPC], DT)
            nc.sync.dma_start(out=corrt_sb[:], in_=corrt.ap())
            invct_sb = const.tile([C, RPC], DT)
            nc.sync.dma_start(out=invct_sb[:], in_=invct.ap())
            w1t_sb = const.tile([C, H], DT)
            nc.sync.dma_start(out=w1t_sb[:], in_=w1t.ap())
            b1c_sb = const.tile([H, 1], DT)
            nc.sync.dma_start(out=b1c_sb[:], in_=b1c.ap())
            w2t_sb = const.tile([H, C], DT)
            nc.sync.dma_start(out=w2t_sb[:], in_=w2t.ap())
            b2x2_sb = const.tile([C, 1], DT)
            nc.sync.dma_start(out=b2x2_sb[:], in_=b2x2.ap())

            import contextlib
            loop_cm = tc.For_i(0, loop_reps, 1) if loop_reps > 1 else contextlib.nullcontext()
            with loop_cm:
                for rep in range(reps):
                    # phase 1: per-range running sum / max over streamed tiles
                    rhs4 = small.tile([C, 2 * RPC], DT)  # cols: avg0, avg1, mx0, mx1
                    FCMAX = chunk_tiles * F
                    for r in range(RPC):
                        m_acc = accs.tile([128, FCMAX], DT, tag="m_acc")
                        nc.vector.memset(m_acc[:], -1e30)
                        if style == 'pe':
                            ps_s = psum_t.tile([1, 512], DT, tag=f"ps_s{r}")
                            nmm = sum(-(-npts * C // 128) // 512 for _, npts in chunks)
                            mmi = 0
                        else:
                            s_acc = accs.tile([128, FCMAX], DT, tag="s_acc")
                            nc.vector.memset(s_acc[:], 0.0)
                        for ci, (off, npts) in enumerate(chunks):
                            if not do_phase1:
                                break
                            fc = npts * C // 128
                            xt = xpool.tile([128, FCMAX], DT, tag="xt")
                            eng = [nc.sync, nc.scalar, nc.gpsimd][ci % dma_engines]
                            eng.dma_start(out=xt[:, :fc],
                                          in_=dram_chunk_ap(xs, r, off, npts))
                            if do_max:
                                meng = (nc.gpsimd if (max_gps_every and
                                        ci % max_gps_every == max_gps_every - 1)
                                        else nc.vector)
                                meng.tensor_max(m_acc[:, :fc], m_acc[:, :fc],
                                                xt[:, :fc])
                            if do_add and style == 'pe':
                                xb = stat.tile([128, FCMAX], mybir.dt.bfloat16,
                                               tag="xb")
                                nc.scalar.copy(out=xb[:, :fc], in_=xt[:, :fc])
                                for j in range(fc // 512):
                                    nc.tensor.matmul(
                                        out=ps_s[:], lhsT=ones_bf[:],
                                        rhs=xb[:, j * 512:(j + 1) * 512],
                                        start=(mmi == 0), stop=(mmi == nmm - 1))
                                    mmi += 1
                            elif do_add:
                                for j in range(fc // F):
                                    nc.vector.tensor_add(
                                        s_acc[:, j * F:(j + 1) * F],
                                        s_acc[:, j * F:(j + 1) * F],
                                        xt[:, j * F:(j + 1) * F])

                        # fold sums
                        sum_col = small.tile([C, 1], DT, tag=f"sum_col{r}")
                        if style == 'pe':
                            sum_row = small.tile([1, C], DT, tag=f"sum_row{r}")
                            nc.vector.reduce_sum(
                                out=sum_row[:],
                                in_=ps_s[:].rearrange("p (a c) -> p c a", c=C),
                                axis=mybir.AxisListType.X)
                            sc_ps = psum_w.tile([C, 1], DT, tag="sc")
                            nc.tensor.transpose(out=sc_ps[:], in_=sum_row[:],
                                                identity=ident[:1, :1])
                            nc.vector.tensor_copy(sum_col[:], sc_ps[:])
                        else:
                            s64 = small.tile([128, C], DT, tag="s64")
                            nc.vector.reduce_sum(
                                out=s64[:],
                                in_=s_acc[:].rearrange("p (a c) -> p c a", c=C),
                                axis=mybir.AxisListType.X)
                            srow_t = psum_t.tile([C, 128], DT, tag="tr")
                            nc.tensor.transpose(out=srow_t[:], in_=s64[:],
                                                identity=ident[:])
                            nc.vector.reduce_sum(out=sum_col[:], in_=srow_t[:],
                                                 axis=mybir.AxisListType.X)

                        # fold max: free fold then partition fold
                        m64 = small.tile([128, C], DT, tag="m64")
                        nc.vector.reduce_max(
                            out=m64[:],
                            in_=m_acc[:].rearrange("p (a c) -> p c a", c=C),
                            axis=mybir.AxisListType.X)
                        mrow_t = psum_t.tile([C, 128], DT, tag="tr")
                        nc.tensor.transpose(out=mrow_t[:], in_=m64[:], identity=ident[:])
                        nc.vector.reduce_max(out=rhs4[:, RPC + r:RPC + r + 1], in_=mrow_t[:],
                                             axis=mybir.AxisListType.X)

                        # avg = (sum - corr) * invc
                        nc.vector.tensor_sub(sum_col[:], sum_col[:], corrt_sb[:, r:r + 1])
                        nc.vector.tensor_mul(rhs4[:, r:r + 1], sum_col[:],
                                             invct_sb[:, r:r + 1])

                    # tiny MLP: att = sigmoid(mlp(avg) + mlp(mx)); scale = 1 + att
                    h_ps = psum_w.tile([H, 2 * RPC], DT, tag="mm")
                    nc.tensor.matmul(out=h_ps[:], lhsT=w1t_sb[:], rhs=rhs4[:],
                                     start=True, stop=True)
                    h_sb = small.tile([H, 2 * RPC], DT)
                    nc.scalar.activation(out=h_sb[:], in_=h_ps[:],
                                         func=mybir.ActivationFunctionType.Relu,
                                         bias=b1c_sb[:])
                    z_ps = psum_w.tile([C, 2 * RPC], DT, tag="mm")
                    nc.tensor.matmul(out=z_ps[:], lhsT=w2t_sb[:], rhs=h_sb[:],
                                     start=True, stop=True)
                    z_sb = small.tile([C, 2 * RPC], DT)
                    nc.vector.tensor_copy(z_sb[:], z_ps[:])
                    zsum = small.tile([C, RPC], DT)
                    nc.vector.tensor_add(zsum[:], z_sb[:, 0:RPC], z_sb[:, RPC:2 * RPC])
                    scale_t = small.tile([C, RPC], DT)
                    nc.scalar.activation(out=scale_t[:], in_=zsum[:],
                                         func=mybir.ActivationFunctionType.Sigmoid,
                                         bias=b2x2_sb[:])
                    nc.vector.tensor_scalar_add(scale_t[:], scale_t[:], 1.0)

                    # broadcast each range's scale column to [128, C]
                    mults = []
                    for r in range(RPC):
                        row_ps = psum_w.tile([1, C], DT, tag="row")
                        nc.tensor.transpose(out=row_ps[:], in_=scale_t[:, r:r + 1],
                                            identity=ident[:C, :C])
                        row_sb = small.tile([1, C], DT, tag=f"row_sb{r}")
                        nc.vector.tensor_copy(row_sb[:], row_ps[:])
                        bcast_ps = psum_w.tile([128, C], DT, tag="bc")
                        nc.tensor.matmul(out=bcast_ps[:], lhsT=ones_row[:], rhs=row_sb[:],
                                         start=True, stop=True)
                        mult = accs.tile([128, C], DT, tag=f"mult{r}")
                        nc.vector.tensor_copy(mult[:], bcast_ps[:])
                        mults.append(mult)

                    # phase 2: out = x * scale[batch]
                    for r in range(RPC):
                        for ci, (off, npts) in enumerate(chunks):
                            if not do_phase2:
                                break
                            fa = npts // 128
                            mult_bc = mults[r][:].unsqueeze(1).to_broadcast(
                                [128, fa, C])
                            xt = xpool.tile([128, chunk_tiles * FA, C], DT, tag="xt")
                            eng_i = [nc.sync, nc.scalar, nc.gpsimd][ci % dma_engines]
                            if out_scalar:
                                eng_o = nc.scalar
                            else:
                                eng_o = [nc.sync, nc.scalar, nc.gpsimd][
                                    (ci + 1) % dma_engines]
                            eng_i.dma_start(out=xt[:, :fa, :],
                                            in_=dram_chunk_ap(xs, r, off, npts))
                            yt = xt if inplace else xpool.tile(
                                [128, chunk_tiles * FA, C], DT, tag="yt")
                            if do_mul:
                                nc.vector.tensor_mul(yt[:, :fa, :], xt[:, :fa, :],
                                                     mult_bc)
                            eng_o.dma_start(out=dram_chunk_ap(out, r, off, npts),
                                            in_=yt[:, :fa, :])

    nc.compile()
    return nc


_CACHE: dict[int, object] = {}


def kernel(x, batch_ids, W1, b1, W2, b2):
    x = np.ascontiguousarray(np.asarray(x, dtype=np.float32))
    batch_ids = np.asarray(batch_ids, dtype=np.int32)
    W1 = np.asarray(W1, dtype=np.float32)
    b1 = np.asarray(b1, dtype=np.float32)
    W2 = np.asarray(W2, dtype=np.float32)
    b2 = np.asarray(b2, dtype=np.float32)

    N = x.shape[0]
    bounds = np.searchsorted(batch_ids, np.arange(B + 1), side="left")
    counts = np.diff(bounds)
    R = max(TP, int(-(-counts.max() // TP)) * TP)

    nc = _CACHE.get(R)
    if nc is None:
        nc = _CACHE[R] = build_nc(R)

    xp = np.empty((NCORES, RPC, R, C), np.float32)
    corrt = np.zeros((NCORES, C, RPC), np.float32)
    invct = np.zeros((NCORES, C, RPC), np.float32)
    for b in range(B):
        core, r = divmod(b, RPC)
        s, e = int(bounds[b]), int(bounds[b + 1])
        n = e - s
        xp[core, r, :n] = x[s:e]
        pad = x[s] if n > 0 else np.zeros(C, np.float32)
        xp[core, r, n:] = pad
        corrt[core, :, r] = np.float64(R - n) * pad.astype(np.float64)
        invct[core, :, r] = 1.0 / max(n, 1)

    w1t = np.ascontiguousarray(W1.T)  # [C, H]
    b1c = np.ascontiguousarray(b1.reshape(H, 1))
    w2t = np.ascontiguousarray(W2.T)  # [H, C]
    b2x2 = np.ascontiguousarray((2.0 * b2).reshape(C, 1))

    in_maps = [
        {
            "xs": xp[core],
            "corrt": np.ascontiguousarray(corrt[core]),
            "invct": np.ascontiguousarray(invct[core]),
            "w1t": w1t,
            "b1c": b1c,
            "w2t": w2t,
            "b2x2": b2x2,
        }
        for core in range(NCORES)
    ]

    res = run_bass_kernel_spmd(nc, in_maps, core_ids=list(range(NCORES)))

    out = np.empty((N, C), np.float32)
    for b in range(B):
        core, r = divmod(b, RPC)
        s, e = int(bounds[b]), int(bounds[b + 1])
        out[s:e] = res.results[core]["out"][r, : e - s]
    return out



# revision 2
# speedup vs baseline: 2.0817x; 2.0817x over previous
"""Trainium2 Bass kernel for ChannelAttentionModel (segment avg/max -> tiny MLP ->
sigmoid gate -> per-point scale), SPMD across 8 NeuronCores.

Sharding: batch_ids is sorted with B=16 segments; core k owns batches 2k and
2k+1 (whole batches per device). Each batch range is padded to a fixed R points
by replicating the first row of the batch (max-safe); the extra rows' sum
contribution is subtracted via a host-computed correction term.

x is shipped and processed in bf16 (tolerance is 2e-2; bf16 keeps worst-case
elementwise error well under 1%), halving both host<->device transfer bytes and
on-device HBM traffic. Stats (sum via PE matmul with a ones vector into f32
PSUM, max via DVE) and the tiny MLP stay in f32.
"""

import sys

for _p in ("/opt/trn_rl_repo", "/root/.axon_site/_ro/trn_rl_repo"):
    if _p not in sys.path:
        sys.path.append(_p)

import numpy as np
import ml_dtypes

import concourse.bacc as bacc
import concourse.tile as tile
from concourse import bass, mybir
from concourse.bass_utils import run_bass_kernel_spmd
from concourse.masks import make_identity

NCORES = 8
B = 16
C = 64
H = 32
RPC = 2  # batch ranges per core
TP = 4096  # points per tile
FA = TP // 128  # free-dim point groups per partition (32)
F = FA * C  # free elems per partition per tile (2048)
DT = mybir.dt.float32
DTX = mybir.dt.bfloat16
NPX = ml_dtypes.bfloat16


def build_nc(R: int, chunk_tiles: int = 4, xbufs: int = 6):
    nc = bacc.Bacc("TRN2", target_bir_lowering=False, debug=False,
                   num_devices=NCORES, enable_asserts=False)

    xs = nc.dram_tensor("xs", [RPC, R, C], DTX, kind="ExternalInput")
    corrt = nc.dram_tensor("corrt", [C, RPC], DT, kind="ExternalInput")
    invct = nc.dram_tensor("invct", [C, RPC], DT, kind="ExternalInput")
    w1t = nc.dram_tensor("w1t", [C, H], DT, kind="ExternalInput")
    b1c = nc.dram_tensor("b1c", [H, 1], DT, kind="ExternalInput")
    w2t = nc.dram_tensor("w2t", [H, C], DT, kind="ExternalInput")
    b2x2 = nc.dram_tensor("b2x2", [C, 1], DT, kind="ExternalInput")
    out = nc.dram_tensor("out", [RPC, R, C], DTX, kind="ExternalOutput")

    def dram_chunk_ap(handle, r, off, npts):
        return handle.ap()[r, off:off + npts, :].rearrange(
            "(p a) c -> p (a c)", p=128)

    # chunk each range into large DMA transfers
    chunks = []
    off = 0
    while off < R:
        npts = min(chunk_tiles * TP, R - off)
        chunks.append((off, npts))
        off += npts

    FCMAX = chunk_tiles * F

    with tile.TileContext(nc) as tc:
        with (
            tc.tile_pool(name="const", bufs=1) as const,
            tc.tile_pool(name="xpool", bufs=xbufs) as xpool,
            tc.tile_pool(name="accs", bufs=1) as accs,
            tc.tile_pool(name="small", bufs=1) as small,
            tc.tile_pool(name="psum_t", bufs=1, space="PSUM") as psum_t,
            tc.tile_pool(name="psum_w", bufs=1, space="PSUM") as psum_w,
        ):
            # constants
            ident = const.tile([128, 128], DT)
            make_identity(nc, ident[:])
            ones_row = const.tile([1, 128], DT)
            nc.vector.memset(ones_row[:], 1.0)
            ones_bf = const.tile([128, 1], mybir.dt.bfloat16)
            nc.vector.memset(ones_bf[:], 1.0)
            corrt_sb = const.tile([C, RPC], DT)
            nc.sync.dma_start(out=corrt_sb[:], in_=corrt.ap())
            invct_sb = const.tile([C, RPC], DT)
            nc.sync.dma_start(out=invct_sb[:], in_=invct.ap())
            w1t_sb = const.tile([C, H], DT)
            nc.sync.dma_start(out=w1t_sb[:], in_=w1t.ap())
            b1c_sb = const.tile([H, 1], DT)
            nc.sync.dma_start(out=b1c_sb[:], in_=b1c.ap())
            w2t_sb = const.tile([H, C], DT)
            nc.sync.dma_start(out=w2t_sb[:], in_=w2t.ap())
            b2x2_sb = const.tile([C, 1], DT)
            nc.sync.dma_start(out=b2x2_sb[:], in_=b2x2.ap())

            # phase 1: per-range running max (DVE) + sum (PE ones-matmul)
            rhs4 = small.tile([C, 2 * RPC], DT)  # cols: avg0, avg1, mx0, mx1
            for r in range(RPC):
                m_acc = accs.tile([128, FCMAX], DTX, tag="m_acc")
                nc.vector.memset(m_acc[:], -1e30)
                ps_s = psum_t.tile([1, 512], DT, tag=f"ps_s{r}")
                nmm = sum(npts * C // 128 // 512 for _, npts in chunks)
                mmi = 0
                for ci, (off, npts) in enumerate(chunks):
                    fc = npts * C // 128
                    xt = xpool.tile([128, FCMAX], DTX, tag="xt")
                    nc.sync.dma_start(out=xt[:, :fc],
                                      in_=dram_chunk_ap(xs, r, off, npts))
                    nc.vector.tensor_max(m_acc[:, :fc], m_acc[:, :fc],
                                         xt[:, :fc])
                    for j in range(fc // 512):
                        nc.tensor.matmul(
                            out=ps_s[:], lhsT=ones_bf[:],
                            rhs=xt[:, j * 512:(j + 1) * 512],
                            start=(mmi == 0), stop=(mmi == nmm - 1))
                        mmi += 1

                # fold sums: [1,512] psum holds (a c) partial sums
                sum_col = small.tile([C, 1], DT, tag=f"sum_col{r}")
                sum_row = small.tile([1, C], DT, tag=f"sum_row{r}")
                nc.vector.reduce_sum(
                    out=sum_row[:],
                    in_=ps_s[:].rearrange("p (a c) -> p c a", c=C),
                    axis=mybir.AxisListType.X)
                sc_ps = psum_w.tile([C, 1], DT, tag="sc")
                nc.tensor.transpose(out=sc_ps[:], in_=sum_row[:],
                                    identity=ident[:1, :1])
                nc.vector.tensor_copy(sum_col[:], sc_ps[:])

                # fold max: free fold then partition fold
                m64 = small.tile([128, C], DT, tag="m64")
                nc.vector.reduce_max(
                    out=m64[:],
                    in_=m_acc[:].rearrange("p (a c) -> p c a", c=C),
                    axis=mybir.AxisListType.X)
                mrow_t = psum_t.tile([C, 128], DT, tag="tr")
                nc.tensor.transpose(out=mrow_t[:], in_=m64[:], identity=ident[:])
                nc.vector.reduce_max(out=rhs4[:, RPC + r:RPC + r + 1],
                                     in_=mrow_t[:], axis=mybir.AxisListType.X)

                # avg = (sum - corr) * invc
                nc.vector.tensor_sub(sum_col[:], sum_col[:], corrt_sb[:, r:r + 1])
                nc.vector.tensor_mul(rhs4[:, r:r + 1], sum_col[:],
                                     invct_sb[:, r:r + 1])

            # tiny MLP: att = sigmoid(mlp(avg) + mlp(mx)); scale = 1 + att
            h_ps = psum_w.tile([H, 2 * RPC], DT, tag="mm")
            nc.tensor.matmul(out=h_ps[:], lhsT=w1t_sb[:], rhs=rhs4[:],
                             start=True, stop=True)
            h_sb = small.tile([H, 2 * RPC], DT)
            nc.scalar.activation(out=h_sb[:], in_=h_ps[:],
                                 func=mybir.ActivationFunctionType.Relu,
                                 bias=b1c_sb[:])
            z_ps = psum_w.tile([C, 2 * RPC], DT, tag="mm")
            nc.tensor.matmul(out=z_ps[:], lhsT=w2t_sb[:], rhs=h_sb[:],
                             start=True, stop=True)
            z_sb = small.tile([C, 2 * RPC], DT)
            nc.vector.tensor_copy(z_sb[:], z_ps[:])
            zsum = small.tile([C, RPC], DT)
            nc.vector.tensor_add(zsum[:], z_sb[:, 0:RPC], z_sb[:, RPC:2 * RPC])
            scale_t = small.tile([C, RPC], DT)
            nc.scalar.activation(out=scale_t[:], in_=zsum[:],
                                 func=mybir.ActivationFunctionType.Sigmoid,
                                 bias=b2x2_sb[:])
            nc.vector.tensor_scalar_add(scale_t[:], scale_t[:], 1.0)

            # broadcast each range's scale column to [128, C]
            mults = []
            for r in range(RPC):
                row_ps = psum_w.tile([1, C], DT, tag="row")
                nc.tensor.transpose(out=row_ps[:], in_=scale_t[:, r:r + 1],
                                    identity=ident[:C, :C])
                row_sb = small.tile([1, C], DT, tag=f"row_sb{r}")
                nc.vector.tensor_copy(row_sb[:], row_ps[:])
                bcast_ps = psum_w.tile([128, C], DT, tag="bc")
                nc.tensor.matmul(out=bcast_ps[:], lhsT=ones_row[:], rhs=row_sb[:],
                                 start=True, stop=True)
                mult = accs.tile([128, C], DT, tag=f"mult{r}")
                nc.vector.tensor_copy(mult[:], bcast_ps[:])
                mults.append(mult)

            # phase 2: out = x * scale[batch]  (load on sync ring, store on
            # scalar ring so the two HWDGE FIFOs run in parallel)
            for r in range(RPC):
                for ci, (off, npts) in enumerate(chunks):
                    fa = npts // 128
                    mult_bc = mults[r][:].unsqueeze(1).to_broadcast(
                        [128, fa, C])
                    xt = xpool.tile([128, chunk_tiles * FA, C], DTX, tag="xt")
                    nc.sync.dma_start(out=xt[:, :fa, :],
                                      in_=dram_chunk_ap(xs, r, off, npts))
                    nc.vector.tensor_mul(xt[:, :fa, :], xt[:, :fa, :], mult_bc)
                    nc.scalar.dma_start(out=dram_chunk_ap(out, r, off, npts),
                                        in_=xt[:, :fa, :])

    nc.compile()
    return nc


_CACHE: dict[int, object] = {}


def kernel(x, batch_ids, W1, b1, W2, b2):
    x = np.ascontiguousarray(np.asarray(x, dtype=np.float32))
    batch_ids = np.asarray(batch_ids, dtype=np.int32)
    W1 = np.asarray(W1, dtype=np.float32)
    b1 = np.asarray(b1, dtype=np.float32)
    W2 = np.asarray(W2, dtype=np.float32)
    b2 = np.asarray(b2, dtype=np.float32)

    N = x.shape[0]
    bounds = np.searchsorted(batch_ids, np.arange(B + 1), side="left")
    counts = np.diff(bounds)
    R = max(TP, int(-(-counts.max() // TP)) * TP)

    nc = _CACHE.get(R)
    if nc is None:
        nc = _CACHE[R] = build_nc(R)

    xb = x.astype(NPX)
    xp = np.empty((NCORES, RPC, R, C), NPX)
    corrt = np.zeros((NCORES, C, RPC), np.float32)
    invct = np.zeros((NCORES, C, RPC), np.float32)
    for b in range(B):
        core, r = divmod(b, RPC)
        s, e = int(bounds[b]), int(bounds[b + 1])
        n = e - s
        xp[core, r, :n] = xb[s:e]
        pad = xb[s] if n > 0 else np.zeros(C, NPX)
        xp[core, r, n:] = pad
        # device sums bf16 values in f32 psum; the pad rows contribute
        # (R - n) * f32(bf16(pad)) exactly
        corrt[core, :, r] = np.float64(R - n) * pad.astype(np.float64)
        invct[core, :, r] = 1.0 / max(n, 1)

    w1t = np.ascontiguousarray(W1.T)  # [C, H]
    b1c = np.ascontiguousarray(b1.reshape(H, 1))
    w2t = np.ascontiguousarray(W2.T)  # [H, C]
    b2x2 = np.ascontiguousarray((2.0 * b2).reshape(C, 1))

    in_maps = [
        {
            "xs": xp[core],
            "corrt": np.ascontiguousarray(corrt[core]),
            "invct": np.ascontiguousarray(invct[core]),
            "w1t": w1t,
            "b1c": b1c,
            "w2t": w2t,
            "b2x2": b2x2,
        }
        for core in range(NCORES)
    ]

    res = run_bass_kernel_spmd(nc, in_maps, core_ids=list(range(NCORES)))

    out = np.empty((N, C), np.float32)
    for b in range(B):
        core, r = divmod(b, RPC)
        s, e = int(bounds[b]), int(bounds[b + 1])
        out[s:e] = res.results[core]["out"][r, : e - s].astype(np.float32)
    return out


# revision 6
# speedup vs baseline: 2.1552x; 1.0353x over previous
"""Trainium2 Bass kernel for ChannelAttentionModel (segment avg/max -> tiny MLP ->
sigmoid gate -> per-point scale), SPMD across 8 NeuronCores.

Sharding: batch_ids is sorted with B=16 segments; core k owns batches 2k and
2k+1 (whole batches per device). Each batch range is padded to a fixed R points
by replicating the first row of the batch (max-safe); the extra rows' sum
contribution is subtracted via a host-computed correction term.

x is shipped and processed in bf16 (tolerance is 2e-2; bf16 keeps worst-case
elementwise error well under 1%), halving both host<->device transfer bytes and
on-device HBM traffic. Stats (sum via PE matmul with a ones vector into f32
PSUM, max via DVE) and the tiny MLP stay in f32.
"""

import sys

for _p in ("/opt/trn_rl_repo", "/root/.axon_site/_ro/trn_rl_repo"):
    if _p not in sys.path:
        sys.path.append(_p)

import numpy as np
import ml_dtypes

import concourse.bacc as bacc
import concourse.tile as tile
from concourse import bass, mybir
from concourse.bass_utils import run_bass_kernel_spmd
from concourse.masks import make_identity

NCORES = 8
B = 16
C = 64
H = 32
RPC = 2  # batch ranges per core
TP = 4096  # points per tile
FA = TP // 128  # free-dim point groups per partition (32)
F = FA * C  # free elems per partition per tile (2048)
DT = mybir.dt.float32
DTX = mybir.dt.bfloat16
NPX = ml_dtypes.bfloat16


def build_nc(R: int, chunk_tiles: int = 4, xbufs: int = 8):
    nc = bacc.Bacc("TRN2", target_bir_lowering=False, debug=False,
                   num_devices=NCORES, enable_asserts=False)

    xs = nc.dram_tensor("xs", [RPC, R, C], DTX, kind="ExternalInput")
    corrt = nc.dram_tensor("corrt", [C, RPC], DT, kind="ExternalInput")
    invct = nc.dram_tensor("invct", [C, RPC], DT, kind="ExternalInput")
    w1t = nc.dram_tensor("w1t", [C, H], DT, kind="ExternalInput")
    b1c = nc.dram_tensor("b1c", [H, 1], DT, kind="ExternalInput")
    w2t = nc.dram_tensor("w2t", [H, C], DT, kind="ExternalInput")
    b2x2 = nc.dram_tensor("b2x2", [C, 1], DT, kind="ExternalInput")
    out = nc.dram_tensor("out", [RPC, R, C], DTX, kind="ExternalOutput")

    def dram_chunk_ap(handle, r, off, npts):
        return handle.ap()[r, off:off + npts, :].rearrange(
            "(p a) c -> p (a c)", p=128)

    # chunk each range into large DMA transfers
    chunks = []
    off = 0
    while off < R:
        npts = min(chunk_tiles * TP, R - off)
        chunks.append((off, npts))
        off += npts

    FCMAX = chunk_tiles * F

    with tile.TileContext(nc) as tc:
        with (
            tc.tile_pool(name="const", bufs=1) as const,
            tc.tile_pool(name="xpool", bufs=xbufs) as xpool,
            tc.tile_pool(name="accs", bufs=1) as accs,
            tc.tile_pool(name="small", bufs=1) as small,
            tc.tile_pool(name="psum_t", bufs=1, space="PSUM") as psum_t,
            tc.tile_pool(name="psum_w", bufs=1, space="PSUM") as psum_w,
        ):
            # constants
            ident = const.tile([128, 128], DT)
            make_identity(nc, ident[:])
            ones_row = const.tile([1, 128], DT)
            nc.vector.memset(ones_row[:], 1.0)
            ones_bf = const.tile([128, 1], mybir.dt.bfloat16)
            nc.vector.memset(ones_bf[:], 1.0)
            # const loads go on the scalar ring so the first x-chunk loads
            # (sync ring, FIFO) start immediately
            corrt_sb = const.tile([C, RPC], DT)
            nc.scalar.dma_start(out=corrt_sb[:], in_=corrt.ap())
            invct_sb = const.tile([C, RPC], DT)
            nc.scalar.dma_start(out=invct_sb[:], in_=invct.ap())
            w1t_sb = const.tile([C, H], DT)
            nc.scalar.dma_start(out=w1t_sb[:], in_=w1t.ap())
            b1c_sb = const.tile([H, 1], DT)
            nc.scalar.dma_start(out=b1c_sb[:], in_=b1c.ap())
            w2t_sb = const.tile([H, C], DT)
            nc.scalar.dma_start(out=w2t_sb[:], in_=w2t.ap())
            b2x2_sb = const.tile([C, 1], DT)
            nc.scalar.dma_start(out=b2x2_sb[:], in_=b2x2.ap())

            # phase 1: per-range running max (DVE) + sum (PE ones-matmul)
            rhs4 = small.tile([C, 2 * RPC], DT)  # cols: avg0, avg1, mx0, mx1
            for r in range(RPC):
                m_acc = accs.tile([128, FCMAX], DTX, tag="m_acc")
                ps_s = psum_t.tile([1, 512], DT, tag=f"ps_s{r}")
                nmm = sum(npts * C // 128 // 512 for _, npts in chunks)
                mmi = 0
                for ci, (off, npts) in enumerate(chunks):
                    fc = npts * C // 128
                    xt = xpool.tile([128, FCMAX], DTX, tag="xt")
                    nc.sync.dma_start(out=xt[:, :fc],
                                      in_=dram_chunk_ap(xs, r, off, npts))
                    if ci == 0:
                        # first chunk is always full-width: init the running
                        # max with a copy (4x bf16 mode) instead of memset+max
                        assert fc == FCMAX
                        nc.vector.tensor_copy(m_acc[:], xt[:])
                    else:
                        nc.vector.tensor_max(m_acc[:, :fc], m_acc[:, :fc],
                                             xt[:, :fc])
                    for j in range(fc // 512):
                        nc.tensor.matmul(
                            out=ps_s[:], lhsT=ones_bf[:],
                            rhs=xt[:, j * 512:(j + 1) * 512],
                            start=(mmi == 0), stop=(mmi == nmm - 1))
                        mmi += 1

                # fold sums: [1,512] psum holds (a c) partial sums
                sum_col = small.tile([C, 1], DT, tag=f"sum_col{r}")
                sum_row = small.tile([1, C], DT, tag=f"sum_row{r}")
                nc.vector.reduce_sum(
                    out=sum_row[:],
                    in_=ps_s[:].rearrange("p (a c) -> p c a", c=C),
                    axis=mybir.AxisListType.X)
                sc_ps = psum_w.tile([C, 1], DT, tag="sc")
                nc.tensor.transpose(out=sc_ps[:], in_=sum_row[:],
                                    identity=ident[:1, :1])
                nc.vector.tensor_copy(sum_col[:], sc_ps[:])

                # fold max: free fold then partition fold
                m64 = small.tile([128, C], DT, tag="m64")
                nc.vector.reduce_max(
                    out=m64[:],
                    in_=m_acc[:].rearrange("p (a c) -> p c a", c=C),
                    axis=mybir.AxisListType.X)
                mrow_t = psum_t.tile([C, 128], DT, tag="tr")
                nc.tensor.transpose(out=mrow_t[:], in_=m64[:], identity=ident[:])
                nc.vector.reduce_max(out=rhs4[:, RPC + r:RPC + r + 1],
                                     in_=mrow_t[:], axis=mybir.AxisListType.X)

                # avg = (sum - corr) * invc
                nc.vector.tensor_sub(sum_col[:], sum_col[:], corrt_sb[:, r:r + 1])
                nc.vector.tensor_mul(rhs4[:, r:r + 1], sum_col[:],
                                     invct_sb[:, r:r + 1])

            # tiny MLP: att = sigmoid(mlp(avg) + mlp(mx)); scale = 1 + att
            h_ps = psum_w.tile([H, 2 * RPC], DT, tag="mm")
            nc.tensor.matmul(out=h_ps[:], lhsT=w1t_sb[:], rhs=rhs4[:],
                             start=True, stop=True)
            h_sb = small.tile([H, 2 * RPC], DT)
            nc.scalar.activation(out=h_sb[:], in_=h_ps[:],
                                 func=mybir.ActivationFunctionType.Relu,
                                 bias=b1c_sb[:])
            z_ps = psum_w.tile([C, 2 * RPC], DT, tag="mm")
            nc.tensor.matmul(out=z_ps[:], lhsT=w2t_sb[:], rhs=h_sb[:],
                             start=True, stop=True)
            z_sb = small.tile([C, 2 * RPC], DT)
            nc.vector.tensor_copy(z_sb[:], z_ps[:])
            zsum = small.tile([C, RPC], DT)
            nc.vector.tensor_add(zsum[:], z_sb[:, 0:RPC], z_sb[:, RPC:2 * RPC])
            scale_t = small.tile([C, RPC], DT)
            nc.scalar.activation(out=scale_t[:], in_=zsum[:],
                                 func=mybir.ActivationFunctionType.Sigmoid,
                                 bias=b2x2_sb[:])
            nc.vector.tensor_scalar_add(scale_t[:], scale_t[:], 1.0)

            # broadcast each range's scale column to [128, C]
            mults = []
            for r in range(RPC):
                row_ps = psum_w.tile([1, C], DT, tag="row")
                nc.tensor.transpose(out=row_ps[:], in_=scale_t[:, r:r + 1],
                                    identity=ident[:C, :C])
                row_sb = small.tile([1, C], DT, tag=f"row_sb{r}")
                nc.vector.tensor_copy(row_sb[:], row_ps[:])
                bcast_ps = psum_w.tile([128, C], DT, tag="bc")
                nc.tensor.matmul(out=bcast_ps[:], lhsT=ones_row[:], rhs=row_sb[:],
                                 start=True, stop=True)
                # bf16 scale tile: all-bf16 tensor_mul in phase 2 gets the
                # DVE 2x perf mode (a f32 operand drops it to 1x)
                mult = accs.tile([128, C], DTX, tag=f"mult{r}")
                nc.vector.tensor_copy(mult[:], bcast_ps[:])
                mults.append(mult)

            # phase 2: out = x * scale[batch]  (load on sync ring, store on
            # scalar ring so the two HWDGE FIFOs run in parallel)
            for r in range(RPC):
                for ci, (off, npts) in enumerate(chunks):
                    fa = npts // 128
                    mult_bc = mults[r][:].unsqueeze(1).to_broadcast(
                        [128, fa, C])
                    xt = xpool.tile([128, chunk_tiles * FA, C], DTX, tag="xt")
                    nc.sync.dma_start(out=xt[:, :fa, :],
                                      in_=dram_chunk_ap(xs, r, off, npts))
                    nc.vector.tensor_mul(xt[:, :fa, :], xt[:, :fa, :], mult_bc)
                    nc.scalar.dma_start(out=dram_chunk_ap(out, r, off, npts),
                                        in_=xt[:, :fa, :])

    nc.compile()
    return nc


_CACHE: dict[int, object] = {}


def kernel(x, batch_ids, W1, b1, W2, b2):
    x = np.ascontiguousarray(np.asarray(x, dtype=np.float32))
    batch_ids = np.asarray(batch_ids, dtype=np.int32)
    W1 = np.asarray(W1, dtype=np.float32)
    b1 = np.asarray(b1, dtype=np.float32)
    W2 = np.asarray(W2, dtype=np.float32)
    b2 = np.asarray(b2, dtype=np.float32)

    N = x.shape[0]
    bounds = np.searchsorted(batch_ids, np.arange(B + 1), side="left")
    counts = np.diff(bounds)
    R = max(TP, int(-(-counts.max() // TP)) * TP)

    nc = _CACHE.get(R)
    if nc is None:
        nc = _CACHE[R] = build_nc(R)

    xb = x.astype(NPX)
    xp = np.empty((NCORES, RPC, R, C), NPX)
    corrt = np.zeros((NCORES, C, RPC), np.float32)
    invct = np.zeros((NCORES, C, RPC), np.float32)
    for b in range(B):
        core, r = divmod(b, RPC)
        s, e = int(bounds[b]), int(bounds[b + 1])
        n = e - s
        xp[core, r, :n] = xb[s:e]
        pad = xb[s] if n > 0 else np.zeros(C, NPX)
        xp[core, r, n:] = pad
        # device sums bf16 values in f32 psum; the pad rows contribute
        # (R - n) * f32(bf16(pad)) exactly
        corrt[core, :, r] = np.float64(R - n) * pad.astype(np.float64)
        invct[core, :, r] = 1.0 / max(n, 1)

    w1t = np.ascontiguousarray(W1.T)  # [C, H]
    b1c = np.ascontiguousarray(b1.reshape(H, 1))
    w2t = np.ascontiguousarray(W2.T)  # [H, C]
    b2x2 = np.ascontiguousarray((2.0 * b2).reshape(C, 1))

    in_maps = [
        {
            "xs": xp[core],
            "corrt": np.ascontiguousarray(corrt[core]),
            "invct": np.ascontiguousarray(invct[core]),
            "w1t": w1t,
            "b1c": b1c,
            "w2t": w2t,
            "b2x2": b2x2,
        }
        for core in range(NCORES)
    ]

    res = run_bass_kernel_spmd(nc, in_maps, core_ids=list(range(NCORES)))

    out = np.empty((N, C), np.float32)
    for b in range(B):
        core, r = divmod(b, RPC)
        s, e = int(bounds[b]), int(bounds[b + 1])
        out[s:e] = res.results[core]["out"][r, : e - s].astype(np.float32)
    return out


# revision 10
# speedup vs baseline: 2.5128x; 1.1659x over previous
"""Trainium2 Bass kernel for ChannelAttentionModel (segment avg/max -> tiny MLP ->
sigmoid gate -> per-point scale), SPMD across 8 NeuronCores.

Sharding: batch_ids is sorted with B=16 segments; core k owns batches 2k and
2k+1 (whole batches per device). Each batch range is padded to a fixed R points
by replicating the first row of the batch (max-safe); the extra rows' sum
contribution is subtracted via a host-computed correction term.

x is shipped and processed in bf16 (tolerance is 2e-2; bf16 keeps worst-case
elementwise error well under 1%), halving both host<->device transfer bytes and
on-device HBM traffic. Stats (sum via PE matmul with a ones vector into f32
PSUM, max via DVE) and the tiny MLP stay in f32.
"""

import sys

for _p in ("/opt/trn_rl_repo", "/root/.axon_site/_ro/trn_rl_repo"):
    if _p not in sys.path:
        sys.path.append(_p)

import numpy as np
import ml_dtypes

import concourse.bacc as bacc
import concourse.tile as tile
from concourse import bass, mybir
from concourse.bass_utils import run_bass_kernel_spmd
from concourse.masks import make_identity

NCORES = 8
B = 16
C = 64
H = 32
RPC = 2  # batch ranges per core
TP = 4096  # points per tile
FA = TP // 128  # free-dim point groups per partition (32)
F = FA * C  # free elems per partition per tile (2048)
DT = mybir.dt.float32
DTX = mybir.dt.bfloat16
NPX = ml_dtypes.bfloat16


def build_nc(R: int, chunk_tiles: int = 4, xbufs: int = 3):
    nc = bacc.Bacc("TRN2", target_bir_lowering=False, debug=False,
                   num_devices=NCORES, enable_asserts=False)

    xs = nc.dram_tensor("xs", [RPC, R, C], DTX, kind="ExternalInput")
    corrt = nc.dram_tensor("corrt", [C, RPC], DT, kind="ExternalInput")
    invct = nc.dram_tensor("invct", [C, RPC], DT, kind="ExternalInput")
    w1t = nc.dram_tensor("w1t", [C, H], DT, kind="ExternalInput")
    b1c = nc.dram_tensor("b1c", [H, 1], DT, kind="ExternalInput")
    w2t = nc.dram_tensor("w2t", [H, C], DT, kind="ExternalInput")
    b2x2 = nc.dram_tensor("b2x2", [C, 1], DT, kind="ExternalInput")
    out = nc.dram_tensor("out", [RPC, R, C], DTX, kind="ExternalOutput")

    def dram_chunk_ap(handle, r, off, npts):
        return handle.ap()[r, off:off + npts, :].rearrange(
            "(p a) c -> p (a c)", p=128)

    # chunk each range into large DMA transfers
    chunks = []
    off = 0
    while off < R:
        npts = min(chunk_tiles * TP, R - off)
        chunks.append((off, npts))
        off += npts

    FCMAX = chunk_tiles * F

    # range 0's chunks stay resident in SBUF between phase 1 and phase 2
    # (skips their phase-2 re-read from HBM). Budget: <=126 KiB/partition.
    res_budget = 126 * 1024
    n_res = 0
    used = 0
    for off, npts in chunks:
        fc_b = npts * C // 128 * 2  # bf16 bytes per partition
        if used + fc_b > res_budget:
            break
        used += fc_b
        n_res += 1

    with tile.TileContext(nc) as tc:
        with (
            tc.tile_pool(name="const", bufs=1) as const,
            tc.tile_pool(name="resp", bufs=1) as resp,
            tc.tile_pool(name="xpool", bufs=xbufs) as xpool,
            tc.tile_pool(name="accs", bufs=1) as accs,
            tc.tile_pool(name="small", bufs=1) as small,
            tc.tile_pool(name="psum_t", bufs=1, space="PSUM") as psum_t,
            tc.tile_pool(name="psum_w", bufs=1, space="PSUM") as psum_w,
        ):
            # constants
            ident = const.tile([128, 128], DT)
            make_identity(nc, ident[:])
            ones_row = const.tile([1, 128], DT)
            nc.vector.memset(ones_row[:], 1.0)
            ones_bf = const.tile([128, 1], mybir.dt.bfloat16)
            nc.vector.memset(ones_bf[:], 1.0)
            # const loads go on the scalar ring so the first x-chunk loads
            # (sync ring, FIFO) start immediately
            corrt_sb = const.tile([C, RPC], DT)
            nc.scalar.dma_start(out=corrt_sb[:], in_=corrt.ap())
            invct_sb = const.tile([C, RPC], DT)
            nc.scalar.dma_start(out=invct_sb[:], in_=invct.ap())
            w1t_sb = const.tile([C, H], DT)
            nc.scalar.dma_start(out=w1t_sb[:], in_=w1t.ap())
            b1c_sb = const.tile([H, 1], DT)
            nc.scalar.dma_start(out=b1c_sb[:], in_=b1c.ap())
            w2t_sb = const.tile([H, C], DT)
            nc.scalar.dma_start(out=w2t_sb[:], in_=w2t.ap())
            b2x2_sb = const.tile([C, 1], DT)
            nc.scalar.dma_start(out=b2x2_sb[:], in_=b2x2.ap())

            # phase 1: per-range running max (DVE) + sum (PE ones-matmul)
            rhs4 = small.tile([C, 2 * RPC], DT)  # cols: avg0, avg1, mx0, mx1
            res_tiles = {}
            for r in range(RPC):
                m_acc = accs.tile([128, FCMAX], DTX, tag="m_acc")
                ps_s = psum_t.tile([1, 512], DT, tag=f"ps_s{r}")
                nmm = sum(npts * C // 128 // 512 for _, npts in chunks)
                mmi = 0
                for ci, (off, npts) in enumerate(chunks):
                    fc = npts * C // 128
                    if r == 0 and ci < n_res:
                        xt = resp.tile([128, fc], DTX, tag=f"res{ci}")
                        res_tiles[ci] = xt
                    else:
                        xt = xpool.tile([128, FCMAX], DTX, tag="xt")
                    nc.sync.dma_start(out=xt[:, :fc],
                                      in_=dram_chunk_ap(xs, r, off, npts))
                    if ci == 0:
                        # first chunk is always full-width: init the running
                        # max with a copy (4x bf16 mode) instead of memset+max
                        assert fc == FCMAX
                        nc.vector.tensor_copy(m_acc[:], xt[:, :fc])
                    else:
                        nc.vector.tensor_max(m_acc[:, :fc], m_acc[:, :fc],
                                             xt[:, :fc])
                    for j in range(fc // 512):
                        nc.tensor.matmul(
                            out=ps_s[:], lhsT=ones_bf[:],
                            rhs=xt[:, j * 512:(j + 1) * 512],
                            start=(mmi == 0), stop=(mmi == nmm - 1))
                        mmi += 1

                # fold sums: [1,512] psum holds (a c) partial sums
                sum_col = small.tile([C, 1], DT, tag=f"sum_col{r}")
                sum_row = small.tile([1, C], DT, tag=f"sum_row{r}")
                nc.vector.reduce_sum(
                    out=sum_row[:],
                    in_=ps_s[:].rearrange("p (a c) -> p c a", c=C),
                    axis=mybir.AxisListType.X)
                sc_ps = psum_w.tile([C, 1], DT, tag="sc")
                nc.tensor.transpose(out=sc_ps[:], in_=sum_row[:],
                                    identity=ident[:1, :1])
                nc.vector.tensor_copy(sum_col[:], sc_ps[:])

                # fold max: free fold then partition fold
                m64 = small.tile([128, C], DT, tag="m64")
                nc.vector.reduce_max(
                    out=m64[:],
                    in_=m_acc[:].rearrange("p (a c) -> p c a", c=C),
                    axis=mybir.AxisListType.X)
                mrow_t = psum_t.tile([C, 128], DT, tag="tr")
                nc.tensor.transpose(out=mrow_t[:], in_=m64[:], identity=ident[:])
                nc.vector.reduce_max(out=rhs4[:, RPC + r:RPC + r + 1],
                                     in_=mrow_t[:], axis=mybir.AxisListType.X)

                # avg = (sum - corr) * invc
                nc.vector.tensor_sub(sum_col[:], sum_col[:], corrt_sb[:, r:r + 1])
                nc.vector.tensor_mul(rhs4[:, r:r + 1], sum_col[:],
                                     invct_sb[:, r:r + 1])

            # tiny MLP: att = sigmoid(mlp(avg) + mlp(mx)); scale = 1 + att
            h_ps = psum_w.tile([H, 2 * RPC], DT, tag="mm")
            nc.tensor.matmul(out=h_ps[:], lhsT=w1t_sb[:], rhs=rhs4[:],
                             start=True, stop=True)
            h_sb = small.tile([H, 2 * RPC], DT)
            nc.scalar.activation(out=h_sb[:], in_=h_ps[:],
                                 func=mybir.ActivationFunctionType.Relu,
                                 bias=b1c_sb[:])
            z_ps = psum_w.tile([C, 2 * RPC], DT, tag="mm")
            nc.tensor.matmul(out=z_ps[:], lhsT=w2t_sb[:], rhs=h_sb[:],
                             start=True, stop=True)
            z_sb = small.tile([C, 2 * RPC], DT)
            nc.vector.tensor_copy(z_sb[:], z_ps[:])
            zsum = small.tile([C, RPC], DT)
            nc.vector.tensor_add(zsum[:], z_sb[:, 0:RPC], z_sb[:, RPC:2 * RPC])
            scale_t = small.tile([C, RPC], DT)
            nc.scalar.activation(out=scale_t[:], in_=zsum[:],
                                 func=mybir.ActivationFunctionType.Sigmoid,
                                 bias=b2x2_sb[:])
            nc.vector.tensor_scalar_add(scale_t[:], scale_t[:], 1.0)

            # broadcast each range's scale column to [128, C]
            mults = []
            for r in range(RPC):
                row_ps = psum_w.tile([1, C], DT, tag="row")
                nc.tensor.transpose(out=row_ps[:], in_=scale_t[:, r:r + 1],
                                    identity=ident[:C, :C])
                row_sb = small.tile([1, C], DT, tag=f"row_sb{r}")
                nc.vector.tensor_copy(row_sb[:], row_ps[:])
                bcast_ps = psum_w.tile([128, C], DT, tag="bc")
                nc.tensor.matmul(out=bcast_ps[:], lhsT=ones_row[:], rhs=row_sb[:],
                                 start=True, stop=True)
                # bf16 scale tile: all-bf16 tensor_mul in phase 2 gets the
                # DVE 2x perf mode (a f32 operand drops it to 1x)
                mult = accs.tile([128, C], DTX, tag=f"mult{r}")
                nc.vector.tensor_copy(mult[:], bcast_ps[:])
                mults.append(mult)

            # phase 2: out = x * scale[batch]  (load on sync ring, store on
            # scalar ring so the two HWDGE FIFOs run in parallel). Range 0's
            # resident chunks need no re-load: multiply in place and store.
            for r in range(RPC):
                for ci, (off, npts) in enumerate(chunks):
                    fa = npts // 128
                    mult_bc = mults[r][:].unsqueeze(1).to_broadcast(
                        [128, fa, C])
                    if r == 0 and ci in res_tiles:
                        xt = res_tiles[ci][:].rearrange(
                            "p (a c) -> p a c", c=C)
                    else:
                        t = xpool.tile([128, chunk_tiles * FA, C], DTX,
                                       tag="xt")
                        xt = t[:, :fa, :]
                        nc.sync.dma_start(out=xt,
                                          in_=dram_chunk_ap(xs, r, off, npts))
                    nc.vector.tensor_mul(xt, xt, mult_bc)
                    nc.scalar.dma_start(out=dram_chunk_ap(out, r, off, npts),
                                        in_=xt)

    nc.compile()
    return nc


_CACHE: dict[int, object] = {}


def kernel(x, batch_ids, W1, b1, W2, b2):
    x = np.ascontiguousarray(np.asarray(x, dtype=np.float32))
    batch_ids = np.asarray(batch_ids, dtype=np.int32)
    W1 = np.asarray(W1, dtype=np.float32)
    b1 = np.asarray(b1, dtype=np.float32)
    W2 = np.asarray(W2, dtype=np.float32)
    b2 = np.asarray(b2, dtype=np.float32)

    N = x.shape[0]
    bounds = np.searchsorted(batch_ids, np.arange(B + 1), side="left")
    counts = np.diff(bounds)
    R = max(TP, int(-(-counts.max() // TP)) * TP)

    nc = _CACHE.get(R)
    if nc is None:
        nc = _CACHE[R] = build_nc(R)

    xb = x.astype(NPX)
    xp = np.empty((NCORES, RPC, R, C), NPX)
    corrt = np.zeros((NCORES, C, RPC), np.float32)
    invct = np.zeros((NCORES, C, RPC), np.float32)
    for b in range(B):
        core, r = divmod(b, RPC)
        s, e = int(bounds[b]), int(bounds[b + 1])
        n = e - s
        xp[core, r, :n] = xb[s:e]
        pad = xb[s] if n > 0 else np.zeros(C, NPX)
        xp[core, r, n:] = pad
        # device sums bf16 values in f32 psum; the pad rows contribute
        # (R - n) * f32(bf16(pad)) exactly
        corrt[core, :, r] = np.float64(R - n) * pad.astype(np.float64)
        invct[core, :, r] = 1.0 / max(n, 1)

    w1t = np.ascontiguousarray(W1.T)  # [C, H]
    b1c = np.ascontiguousarray(b1.reshape(H, 1))
    w2t = np.ascontiguousarray(W2.T)  # [H, C]
    b2x2 = np.ascontiguousarray((2.0 * b2).reshape(C, 1))

    in_maps = [
        {
            "xs": xp[core],
            "corrt": np.ascontiguousarray(corrt[core]),
            "invct": np.ascontiguousarray(invct[core]),
            "w1t": w1t,
            "b1c": b1c,
            "w2t": w2t,
            "b2x2": b2x2,
        }
        for core in range(NCORES)
    ]

    res = run_bass_kernel_spmd(nc, in_maps, core_ids=list(range(NCORES)))

    out = np.empty((N, C), np.float32)
    for b in range(B):
        core, r = divmod(b, RPC)
        s, e = int(bounds[b]), int(bounds[b + 1])
        out[s:e] = res.results[core]["out"][r, : e - s].astype(np.float32)
    return out


# revision 13
# speedup vs baseline: 2.6777x; 1.0656x over previous
"""Trainium2 Bass kernel for ChannelAttentionModel (segment avg/max -> tiny MLP ->
sigmoid gate -> per-point scale), SPMD across 8 NeuronCores.

Sharding: batch_ids is sorted with B=16 segments; core k owns batches 2k and
2k+1 (whole batches per device). Each batch range is padded to a fixed R points
by replicating the first row of the batch (max-safe); the extra rows' sum
contribution is subtracted via a host-computed correction term.

x is shipped and processed in bf16 (tolerance is 2e-2; bf16 keeps worst-case
elementwise error well under 1%), halving both host<->device transfer bytes and
on-device HBM traffic. Stats (sum via PE matmul with a ones vector into f32
PSUM, max via DVE) and the tiny MLP stay in f32.
"""

import sys

for _p in ("/opt/trn_rl_repo", "/root/.axon_site/_ro/trn_rl_repo"):
    if _p not in sys.path:
        sys.path.append(_p)

import numpy as np
import ml_dtypes

import concourse.bacc as bacc
import concourse.tile as tile
from concourse import bass, mybir
from concourse.bass_utils import run_bass_kernel_spmd
from concourse.masks import make_identity

NCORES = 8
B = 16
C = 64
H = 32
RPC = 2  # batch ranges per core
TP = 4096  # points per tile
FA = TP // 128  # free-dim point groups per partition (32)
F = FA * C  # free elems per partition per tile (2048)
DT = mybir.dt.float32
DTX = mybir.dt.bfloat16
NPX = ml_dtypes.bfloat16


def build_nc(R: int, chunk_tiles: int = 4, xbufs: int = 3):
    nc = bacc.Bacc("TRN2", target_bir_lowering=False, debug=False,
                   num_devices=NCORES, enable_asserts=False)

    xs = nc.dram_tensor("xs", [RPC, R, C], DTX, kind="ExternalInput")
    corrt = nc.dram_tensor("corrt", [C, RPC], DT, kind="ExternalInput")
    invct = nc.dram_tensor("invct", [C, RPC], DT, kind="ExternalInput")
    w1t = nc.dram_tensor("w1t", [C, H], DT, kind="ExternalInput")
    b1c = nc.dram_tensor("b1c", [H, 1], DT, kind="ExternalInput")
    w2t = nc.dram_tensor("w2t", [H, C], DT, kind="ExternalInput")
    b2x2 = nc.dram_tensor("b2x2", [C, 1], DT, kind="ExternalInput")
    out = nc.dram_tensor("out", [RPC, R, C], DTX, kind="ExternalOutput")

    def dram_chunk_ap(handle, r, off, npts):
        return handle.ap()[r, off:off + npts, :].rearrange(
            "(p a) c -> p (a c)", p=128)

    # chunk each range into large DMA transfers
    chunks = []
    off = 0
    while off < R:
        npts = min(chunk_tiles * TP, R - off)
        chunks.append((off, npts))
        off += npts

    FCMAX = chunk_tiles * F

    # range 0's chunks stay resident in SBUF between phase 1 and phase 2
    # (skips their phase-2 re-read from HBM). Budget: <=126 KiB/partition.
    res_budget = 126 * 1024
    n_res = 0
    used = 0
    for off, npts in chunks:
        fc_b = npts * C // 128 * 2  # bf16 bytes per partition
        if used + fc_b > res_budget:
            break
        used += fc_b
        n_res += 1

    with tile.TileContext(nc) as tc:
        with (
            tc.tile_pool(name="const", bufs=1) as const,
            tc.tile_pool(name="resp", bufs=1) as resp,
            tc.tile_pool(name="xpool", bufs=xbufs) as xpool,
            tc.tile_pool(name="accs", bufs=1) as accs,
            tc.tile_pool(name="small", bufs=1) as small,
            tc.tile_pool(name="psum_t", bufs=1, space="PSUM") as psum_t,
            tc.tile_pool(name="psum_w", bufs=1, space="PSUM") as psum_w,
        ):
            # constants
            ident = const.tile([128, 128], DT)
            make_identity(nc, ident[:])
            ones_row = const.tile([1, 128], DT)
            nc.vector.memset(ones_row[:], 1.0)
            ones_bf = const.tile([128, 1], mybir.dt.bfloat16)
            nc.vector.memset(ones_bf[:], 1.0)
            # const loads go on the scalar ring so the first x-chunk loads
            # (sync ring, FIFO) start immediately
            corrt_sb = const.tile([C, RPC], DT)
            nc.scalar.dma_start(out=corrt_sb[:], in_=corrt.ap())
            invct_sb = const.tile([C, RPC], DT)
            nc.scalar.dma_start(out=invct_sb[:], in_=invct.ap())
            w1t_sb = const.tile([C, H], DT)
            nc.scalar.dma_start(out=w1t_sb[:], in_=w1t.ap())
            b1c_sb = const.tile([H, 1], DT)
            nc.scalar.dma_start(out=b1c_sb[:], in_=b1c.ap())
            w2t_sb = const.tile([H, C], DT)
            nc.scalar.dma_start(out=w2t_sb[:], in_=w2t.ap())
            b2x2_sb = const.tile([C, 1], DT)
            nc.scalar.dma_start(out=b2x2_sb[:], in_=b2x2.ap())

            # phase 1: per-range running max (DVE) + sum (PE ones-matmul)
            rhs4 = small.tile([C, 2 * RPC], DT)  # cols: avg0, avg1, mx0, mx1
            res_tiles = {}
            for r in range(RPC):
                m_acc = accs.tile([128, FCMAX], DTX, tag="m_acc")
                ps_s = psum_t.tile([1, 512], DT, tag=f"ps_s{r}")
                nmm = sum(npts * C // 128 // 512 for _, npts in chunks)
                mmi = 0
                for ci, (off, npts) in enumerate(chunks):
                    fc = npts * C // 128
                    if r == 0 and ci < n_res:
                        xt = resp.tile([128, fc], DTX, tag=f"res{ci}")
                        res_tiles[ci] = xt
                    else:
                        xt = xpool.tile([128, FCMAX], DTX, tag="xt")
                    nc.sync.dma_start(out=xt[:, :fc],
                                      in_=dram_chunk_ap(xs, r, off, npts))
                    if ci == 0:
                        # first chunk is always full-width: init the running
                        # max with a copy (4x bf16 mode) instead of memset+max
                        assert fc == FCMAX
                        nc.vector.tensor_copy(m_acc[:], xt[:, :fc])
                    else:
                        nc.vector.tensor_max(m_acc[:, :fc], m_acc[:, :fc],
                                             xt[:, :fc])
                    for j in range(fc // 512):
                        nc.tensor.matmul(
                            out=ps_s[:], lhsT=ones_bf[:],
                            rhs=xt[:, j * 512:(j + 1) * 512],
                            start=(mmi == 0), stop=(mmi == nmm - 1))
                        mmi += 1

                # fold sums: [1,512] psum holds (a c) partial sums
                sum_col = small.tile([C, 1], DT, tag=f"sum_col{r}")
                sum_row = small.tile([1, C], DT, tag=f"sum_row{r}")
                nc.vector.reduce_sum(
                    out=sum_row[:],
                    in_=ps_s[:].rearrange("p (a c) -> p c a", c=C),
                    axis=mybir.AxisListType.X)
                sc_ps = psum_w.tile([C, 1], DT, tag="sc")
                nc.tensor.transpose(out=sc_ps[:], in_=sum_row[:],
                                    identity=ident[:1, :1])
                nc.vector.tensor_copy(sum_col[:], sc_ps[:])

                # fold max: in-place binary halvings (2x bf16 tensor_tensor)
                # down to a=16, then a short 1x reduce. Much faster than one
                # big 1x reduce, and releases m_acc for the next range sooner.
                fc_cur = FCMAX
                while fc_cur > C * 16:
                    h = fc_cur // 2
                    nc.vector.tensor_max(m_acc[:, :h], m_acc[:, :h],
                                         m_acc[:, h:fc_cur])
                    fc_cur = h
                m64 = small.tile([128, C], DT, tag="m64")
                nc.vector.reduce_max(
                    out=m64[:],
                    in_=m_acc[:, :fc_cur].rearrange("p (a c) -> p c a", c=C),
                    axis=mybir.AxisListType.X)
                mrow_t = psum_t.tile([C, 128], DT, tag="tr")
                nc.tensor.transpose(out=mrow_t[:], in_=m64[:], identity=ident[:])
                nc.vector.reduce_max(out=rhs4[:, RPC + r:RPC + r + 1],
                                     in_=mrow_t[:], axis=mybir.AxisListType.X)

                # avg = (sum - corr) * invc
                nc.vector.tensor_sub(sum_col[:], sum_col[:], corrt_sb[:, r:r + 1])
                nc.vector.tensor_mul(rhs4[:, r:r + 1], sum_col[:],
                                     invct_sb[:, r:r + 1])

            # tiny MLP: att = sigmoid(mlp(avg) + mlp(mx)); scale = 1 + att
            h_ps = psum_w.tile([H, 2 * RPC], DT, tag="mm")
            nc.tensor.matmul(out=h_ps[:], lhsT=w1t_sb[:], rhs=rhs4[:],
                             start=True, stop=True)
            h_sb = small.tile([H, 2 * RPC], DT)
            nc.scalar.activation(out=h_sb[:], in_=h_ps[:],
                                 func=mybir.ActivationFunctionType.Relu,
                                 bias=b1c_sb[:])
            z_ps = psum_w.tile([C, 2 * RPC], DT, tag="mm")
            nc.tensor.matmul(out=z_ps[:], lhsT=w2t_sb[:], rhs=h_sb[:],
                             start=True, stop=True)
            z_sb = small.tile([C, 2 * RPC], DT)
            nc.vector.tensor_copy(z_sb[:], z_ps[:])
            zsum = small.tile([C, RPC], DT)
            nc.vector.tensor_add(zsum[:], z_sb[:, 0:RPC], z_sb[:, RPC:2 * RPC])
            scale_t = small.tile([C, RPC], DT)
            nc.scalar.activation(out=scale_t[:], in_=zsum[:],
                                 func=mybir.ActivationFunctionType.Sigmoid,
                                 bias=b2x2_sb[:])
            nc.vector.tensor_scalar_add(scale_t[:], scale_t[:], 1.0)

            # broadcast each range's scale column to [128, C]
            mults = []
            for r in range(RPC):
                row_ps = psum_w.tile([1, C], DT, tag="row")
                nc.tensor.transpose(out=row_ps[:], in_=scale_t[:, r:r + 1],
                                    identity=ident[:C, :C])
                row_sb = small.tile([1, C], DT, tag=f"row_sb{r}")
                nc.vector.tensor_copy(row_sb[:], row_ps[:])
                bcast_ps = psum_w.tile([128, C], DT, tag="bc")
                nc.tensor.matmul(out=bcast_ps[:], lhsT=ones_row[:], rhs=row_sb[:],
                                 start=True, stop=True)
                # bf16 scale tile: all-bf16 tensor_mul in phase 2 gets the
                # DVE 2x perf mode (a f32 operand drops it to 1x)
                mult = accs.tile([128, C], DTX, tag=f"mult{r}")
                nc.vector.tensor_copy(mult[:], bcast_ps[:])
                mults.append(mult)

            # phase 2: out = x * scale[batch]  (load on sync ring, store on
            # scalar ring so the two HWDGE FIFOs run in parallel). Range 0's
            # resident chunks need no re-load: multiply in place and store.
            # Interleave streamed (r1) and resident (r0) chunks so the DVE
            # muls recycle the streaming slots steadily.
            order = []
            for ci in range(len(chunks)):
                order.append((1, ci))
                order.append((0, ci))
            for r, ci in order:
                off, npts = chunks[ci]
                fa = npts // 128
                mult_bc = mults[r][:].unsqueeze(1).to_broadcast(
                    [128, fa, C])
                if r == 0 and ci in res_tiles:
                    xt = res_tiles[ci][:].rearrange("p (a c) -> p a c", c=C)
                else:
                    t = xpool.tile([128, chunk_tiles * FA, C], DTX, tag="xt")
                    xt = t[:, :fa, :]
                    nc.sync.dma_start(out=xt,
                                      in_=dram_chunk_ap(xs, r, off, npts))
                nc.vector.tensor_mul(xt, xt, mult_bc)
                nc.scalar.dma_start(out=dram_chunk_ap(out, r, off, npts),
                                    in_=xt)

    nc.compile()
    return nc


_CACHE: dict[int, object] = {}


def kernel(x, batch_ids, W1, b1, W2, b2):
    x = np.ascontiguousarray(np.asarray(x, dtype=np.float32))
    batch_ids = np.asarray(batch_ids, dtype=np.int32)
    W1 = np.asarray(W1, dtype=np.float32)
    b1 = np.asarray(b1, dtype=np.float32)
    W2 = np.asarray(W2, dtype=np.float32)
    b2 = np.asarray(b2, dtype=np.float32)

    N = x.shape[0]
    bounds = np.searchsorted(batch_ids, np.arange(B + 1), side="left")
    counts = np.diff(bounds)
    R = max(TP, int(-(-counts.max() // TP)) * TP)

    nc = _CACHE.get(R)
    if nc is None:
        nc = _CACHE[R] = build_nc(R)

    xb = x.astype(NPX)
    xp = np.empty((NCORES, RPC, R, C), NPX)
    corrt = np.zeros((NCORES, C, RPC), np.float32)
    invct = np.zeros((NCORES, C, RPC), np.float32)
    for b in range(B):
        core, r = divmod(b, RPC)
        s, e = int(bounds[b]), int(bounds[b + 1])
        n = e - s
        xp[core, r, :n] = xb[s:e]
        pad = xb[s] if n > 0 else np.zeros(C, NPX)
        xp[core, r, n:] = pad
        # device sums bf16 values in f32 psum; the pad rows contribute
        # (R - n) * f32(bf16(pad)) exactly
        corrt[core, :, r] = np.float64(R - n) * pad.astype(np.float64)
        invct[core, :, r] = 1.0 / max(n, 1)

    w1t = np.ascontiguousarray(W1.T)  # [C, H]
    b1c = np.ascontiguousarray(b1.reshape(H, 1))
    w2t = np.ascontiguousarray(W2.T)  # [H, C]
    b2x2 = np.ascontiguousarray((2.0 * b2).reshape(C, 1))

    in_maps = [
        {
            "xs": xp[core],
            "corrt": np.ascontiguousarray(corrt[core]),
            "invct": np.ascontiguousarray(invct[core]),
            "w1t": w1t,
            "b1c": b1c,
            "w2t": w2t,
            "b2x2": b2x2,
        }
        for core in range(NCORES)
    ]

    res = run_bass_kernel_spmd(nc, in_maps, core_ids=list(range(NCORES)))

    out = np.empty((N, C), np.float32)
    for b in range(B):
        core, r = divmod(b, RPC)
        s, e = int(bounds[b]), int(bounds[b + 1])
        out[s:e] = res.results[core]["out"][r, : e - s].astype(np.float32)
    return out
